# revision 1
# baseline (speedup 1.0000x reference)
"""Trainium2 Bass kernel for nn_BiLSTM pairwise-scores problem.

Math (reference):
  vec  = concat(word_emb[wi], pos_emb[pi], ext_emb[ei])          [512, 425]
  h    = concat(lstm_cell_f(vec), lstm_cell_b(vec))              [512, 200]
  cat  = [h, vec] for t <= 255 else [vec, h]                     [512, 625]
  f    = cat @ w_mlp_in.T + b_mlp_in                             [512, 400]
  out  = tanh((f[:,None,:] + f[None,:,:]) @ w_mlp_out.T + b_out) [512, 512, 42]

Key factorization: (f_i + f_j) @ W.T + b = g'_i + g'_j with
g' = f @ W.T + b/2, so the O(n^2 * 400 * 42) matmul collapses to a
[512, 42] projection plus a pairwise broadcast-add, implemented on the PE
as a single K=43 matmul per output chunk: lhsT = [g'_i rows; ones row],
rhs = [periodic identity rows; g'_j flattened row].

Sharding: 8 cores = 4 i-blocks (128 rows) x 2 j-halves (256 cols).
Each core runs an identical (SPMD) program on a permuted 384-token slice:
cols 0:128 = its i-block tokens, cols 128:384 = its j-half tokens.
The embedding gather and weight layout (transposes / gate stacking /
hv-vs-vh row ordering) happen on the host; all dense compute (LSTM cells,
MLPs, pairwise + tanh) runs on device in bf16 with fp32 PSUM accumulation.
"""

import os
import sys

import numpy as np

for _p in ("/opt/trn_rl_repo", "/root/.axon_site/_ro/trn_rl_repo"):
    if os.path.isdir(_p) and _p not in sys.path:
        sys.path.insert(0, _p)

import ml_dtypes  # noqa: E402

import concourse.bacc as bacc  # noqa: E402
import concourse.bass as bass  # noqa: E402
import concourse.mybir as mybir  # noqa: E402
from concourse.bass_utils import run_bass_kernel_spmd  # noqa: E402
from concourse.tile import TileContext  # noqa: E402

BF16 = mybir.dt.bfloat16
F32 = mybir.dt.float32
AF = mybir.ActivationFunctionType

SEQ = 512
D_VEC = 425  # 100 + 25 + 300
NREL = 42
T = 384  # per-core tokens: 128 (i-block) + 256 (j-half)
NFLAT = 256 * NREL  # 10752 = per-core output row length
N_CHUNK = 512
N_CHUNKS = NFLAT // N_CHUNK  # 21
GRP = 4  # pairwise chunks fused per PSUM group / tanh / DMA
IC_PER = 16 * NREL  # 672: replication period for the identity pattern

# K-dim tiling of the 425-dim feature axis
KS = [(0, 128), (128, 256), (256, 384), (384, 425)]
# gate order in the stacked [425, 600] gate weight: i_f g_f o_f i_b g_b o_b
GATE_FUNCS = [AF.Sigmoid, AF.Tanh, AF.Sigmoid] * 2
# PSUM slot split between the gate stream (pg) and mlp_in (pf): the gate
# stream needs 4 slots to stay dense past the first ACT evacuation (which
# keeps the PE clock-warmup lottery winnable); mlp_in tolerates 2.
PG_BUFS = 5 if os.environ.get("KV_PSUM2") == "E" else 4
PF_BUFS = 3 if os.environ.get("KV_PSUM2") == "E" else 2
PNAT_TAG = "pf" if os.environ.get("KV_PSUM2") == "E" else "pnat"

# ---- packed bf16 constant layout: [128, NPK] ----
_SEGS = []  # name -> (rows, col_off, width)


def _seg(name, rows, width):
    off = _SEGS[-1][2] + _SEGS[-1][3] if _SEGS else 0
    _SEGS.append((name, rows, off, width))


# interleaved (vt_k, g6_k) pairs so the first gate matmuls can start as
# soon as the first small DMA lands — the gate matmul stream itself then
# warms the PE clock (no dummy warmup needed).
for _k, (_a, _b) in enumerate(KS):
    _seg(f"vt{_k}", _b - _a, 384)
    _seg(f"g6{_k}", _b - _a, 600)
for _g in range(2):
    for _a2 in range(2):
        _seg(f"wh{_g}{_a2}", 100, 400)
for _g in range(2):
    for _k, (_a, _b) in enumerate(KS):
        _seg(f"wv{_g}{_k}", _b - _a, 400)
_seg("wo", 101, 4 * NREL)
_seg("ic", NREL, IC_PER)
SEG = {s[0]: s for s in _SEGS}
NPK = _SEGS[-1][2] + _SEGS[-1][3]
# input DMA split points: one per (vt_k, g6_k) pair, then the remainder
PK_CUTS = [SEG[f"g6{_k}"][2] + SEG[f"g6{_k}"][3] for _k in range(4)] + [NPK]


def _build_program():
    nc = bacc.Bacc()

    pk_d = nc.dram_tensor("pk", [128, NPK], BF16, kind="ExternalInput")
    bias_d = nc.dram_tensor("bias", [100, 11], F32, kind="ExternalInput")
    out_d = nc.dram_tensor("out", [128, NFLAT], F32, kind="ExternalOutput")

    with TileContext(nc) as tc:
        with (
            tc.tile_pool(name="const", bufs=1) as cp,
            tc.tile_pool(name="work", bufs=3) as wp,
            tc.tile_pool(name="outp", bufs=5) as op_,
        ):
            # -------- early on-chip init (no DMA deps) --------
            # bias DMA first on the scalar queue: its ~2us completion
            # receipt must not gate the first gate activation (which gates
            # the pg PSUM rotation and thus the gate matmul density).
            wsrc = cp.tile([128, N_CHUNK], BF16, tag="wsrc")
            nc.gpsimd.memset(wsrc, 0.0)
            # lhsT of the pairwise matmul: rows 0:42 = g'_i, row 42 = 1.0.
            # DVE partition base must be 32-aligned, so memset 32:43 and let
            # the later g' write overwrite rows 32:42.
            el = cp.tile([NREL + 1, 128], BF16, tag="el")
            nc.vector.memset(el[32 : NREL + 1, :], 1.0)
            # warmup activations absorb the ACT table-set load early
            warm2 = cp.tile([1, 8], F32, tag="warm2")
            nc.scalar.activation(out=warm2, in_=wsrc[0:1, 0:8], func=AF.Sigmoid)
            nc.scalar.activation(out=warm2, in_=wsrc[0:1, 0:8], func=AF.Tanh)

            # -------- input DMAs (paired packed chunks + bias) --------
            # The gate matmul stream doubles as the PE clock warmup, so the
            # (vt_k, g6_k) pairs are DMA'd individually to land ASAP.
            # bias rides the scalar engine's HWDGE queue so it neither
            # waits behind nor delays the pk stream on sync
            bias = cp.tile([100, 11], F32, tag="bias")
            nc.scalar.dma_start(out=bias, in_=bias_d[:, :])
            pk = cp.tile([128, NPK], BF16, tag="pk")
            prev = 0
            for cut in PK_CUTS[:4]:
                nc.sync.dma_start(out=pk[:, prev:cut], in_=pk_d[:, prev:cut])
                prev = cut
            nc.sync.dma_start(out=pk[:, prev:NPK], in_=pk_d[:, prev:NPK])

            def seg(name):
                _, rows, off, width = SEG[name]
                return pk[0:rows, off : off + width]

            vt = [seg(f"vt{k}") for k in range(4)]
            g6 = [seg(f"g6{k}") for k in range(4)]
            wh = [[seg(f"wh{g}{a}") for a in range(2)] for g in range(2)]
            wv = [[seg(f"wv{g}{k}") for k in range(4)] for g in range(2)]
            wo = seg("wo")
            ic = seg("ic")

            # pairwise rhs: rows 0:42 = periodic identity, row 42 = g'_j flat
            rr = cp.tile([NREL + 1, NFLAT], BF16, tag="rr")
            ic_rep = bass.AP(
                tensor=ic.tensor,
                offset=ic.offset,
                ap=[ic.ap[0], [0, NFLAT // IC_PER], ic.ap[1]],
            )
            nc.sync.dma_start(out=rr[0:NREL, :], in_=ic_rep)

            with tc.tile_pool(name="psum_pre", bufs=1, space="PSUM") as pp:
                # -------- LSTM gates (both dirs, f-gate skipped) --------
                # Per-direction ordering: i, g (then c = sig(i)*tanh(g) and
                # tanh(c) start immediately), then o, then h — shortens the
                # serial ACT chain to each direction's h.
                def gate(m):
                    pg = pp.tile([100, T], F32, tag="pg", bufs=PG_BUFS, name=f"pg{m}")
                    for k in range(4):
                        nc.tensor.matmul(
                            pg,
                            lhsT=g6[k][:, m * 100 : (m + 1) * 100],
                            rhs=vt[k],
                            start=(k == 0),
                            stop=(k == 3),
                        )
                    a_ = wp.tile([100, T], BF16, tag=f"act{m}", name=f"act{m}")
                    nc.scalar.activation(
                        out=a_,
                        in_=pg,
                        func=GATE_FUNCS[m],
                        bias=bias[0:100, m : m + 1],
                        scale=1.0,
                    )
                    return a_

                acts = [None] * 6
                hh = []
                for d in range(2):
                    si = gate(3 * d)
                    tg = gate(3 * d + 1)
                    c_ = wp.tile([100, T], BF16, tag=f"c{d}")
                    nc.vector.tensor_mul(c_, si, tg)
                    tc_ = wp.tile([100, T], BF16, tag=f"tc{d}")
                    nc.scalar.activation(out=tc_, in_=c_, func=AF.Tanh)
                    so = gate(3 * d + 2)
                    h_ = cp.tile([100, T], BF16, tag=f"h{d}")
                    nc.vector.tensor_mul(h_, so, tc_)
                    hh.append(h_)
                    acts[3 * d : 3 * d + 3] = [si, tg, so]
                # fillers pinned into the LSTM ACT/DVE gap: keep the PE
                # activity monitor from re-throttling the clock. Gated on
                # the last gate activation so the scheduler can't hoist
                # them ahead of the gap.
                pfill = pp.tile([100, T], F32, tag="pg", bufs=PG_BUFS, name="pfill")
                for _ in range(6):
                    nc.tensor.matmul(
                        pfill,
                        lhsT=acts[5][:, 0:100],
                        rhs=acts[5],
                        start=True,
                        stop=True,
                    )

                # -------- mlp_in: fT [400, 384] --------
                # vec pieces first (no h dependency), h pieces close the
                # accumulation group so the PE overlaps the LSTM ACT chain.
                groups = [(0, 0, 128), (1, 128, 384)]  # (g, col_a, col_b)
                fm = []
                for m in range(4):
                    ms = slice(m * 100, (m + 1) * 100)
                    pf = pp.tile([100, T], F32, tag="pf", bufs=PF_BUFS)
                    for g, ca, cb in groups:
                        for k in range(4):
                            nc.tensor.matmul(
                                pf[:, ca:cb],
                                lhsT=wv[g][k][:, ms],
                                rhs=vt[k][:, ca:cb],
                                start=(k == 0),
                                stop=False,
                            )
                        for a in range(2):
                            nc.tensor.matmul(
                                pf[:, ca:cb],
                                lhsT=wh[g][a][:, ms],
                                rhs=hh[a][:, ca:cb],
                                start=False,
                                stop=(a == 1),
                            )
                    # fm[3] carries an extra all-ones row 100 so the natural-
                    # layout mlp_out below can fold +b_out/2 in as a rank-1
                    # term (wo row 100 holds b_out/2). Memset base must be
                    # 32-aligned: set 96:101, rows 96:100 overwritten below.
                    rows = 101 if m == 3 else 100
                    f_ = cp.tile([rows, T], BF16, tag=f"f{m}")
                    if m == 3:
                        nc.vector.memset(f_[96:101, :], 1.0)
                    nc.vector.tensor_scalar_add(
                        f_[0:100, :], pf, bias[0:100, 6 + m : 7 + m]
                    )
                    fm.append(f_)

                # -------- mlp_out, i-block: g'T [42, 128] (+ b_out/2) ----
                pl = pp.tile([NREL, 128], F32, tag="pg", bufs=PG_BUFS, name="pl")
                for m in range(4):
                    nc.tensor.matmul(
                        pl,
                        lhsT=wo[0:100, m * NREL : (m + 1) * NREL],
                        rhs=fm[m][0:100, 0:128],
                        start=(m == 0),
                        stop=(m == 3),
                    )
                nc.vector.tensor_scalar_add(
                    el[0:NREL, :], pl, bias[0:NREL, 10:11]
                )

                # -------- mlp_out, j-half: g' in natural layout ----------
                # g'_nat[t, r] = sum_f fT[f, t] * WoutT[f, r] (+ ones * b/2)
                # — fT is already [f, t], so no transposes are needed; the
                # flatten DMA reads the natural-layout tile partition-major.
                for c in range(2):
                    krows = [100, 100, 100, 101]
                    png = pp.tile([128, NREL], F32, tag=PNAT_TAG, bufs=(PF_BUFS if PNAT_TAG == "pf" else 2), name=f"png{c}")
                    for m in range(4):
                        kr = krows[m]
                        nc.tensor.matmul(
                            png,
                            lhsT=fm[m][0:kr, 128 + c * 128 : 256 + c * 128],
                            rhs=wo[0:kr, m * NREL : (m + 1) * NREL],
                            start=(m == 0),
                            stop=(m == 3),
                        )
                    tj = wp.tile([128, NREL], BF16, tag="tj")
                    nc.vector.tensor_copy(tj, png)
                    nc.sync.dma_start(
                        out=rr[NREL : NREL + 1, c * 128 * NREL : (c + 1) * 128 * NREL],
                        in_=tj,
                    )
                # fillers pinned into the flatten latency gap
                pfill3 = pp.tile([NREL, N_CHUNK], F32, tag="pg", bufs=PG_BUFS, name="pfill3")
                for _ in range(6):
                    nc.tensor.matmul(
                        pfill3,
                        lhsT=tj[:, 0:NREL],
                        rhs=wsrc,
                        start=True,
                        stop=True,
                    )

            # -------- pairwise: tanh(g'_i + g'_j) --------
            # Group sizes: small first group lets the (bottleneck) ACT
            # tanh stream start early; small last group keeps the tail
            # DMA short. Total ACT overhead is identical to uniform 4s.
            grp_plan = (2, 4, 4, 4, 4, 3)
            with tc.tile_pool(name="psum_pair", bufs=2, space="PSUM") as pq:
                c = 0
                for nch in grp_plan:
                    ppair = pq.tile([128, GRP * N_CHUNK], F32, tag="ppair")
                    base = c * N_CHUNK
                    for q in range(nch):
                        nc.tensor.matmul(
                            ppair[:, q * N_CHUNK : (q + 1) * N_CHUNK],
                            lhsT=el,
                            rhs=rr[:, (c + q) * N_CHUNK : (c + q + 1) * N_CHUNK],
                            start=True,
                            stop=True,
                        )
                    ot = op_.tile([128, GRP * N_CHUNK], F32, tag="ot")
                    nc.scalar.activation(
                        out=ot[:, 0 : nch * N_CHUNK],
                        in_=ppair[:, 0 : nch * N_CHUNK],
                        func=AF.Tanh,
                    )
                    nc.sync.dma_start(
                        out=out_d[:, base : base + nch * N_CHUNK],
                        in_=ot[:, 0 : nch * N_CHUNK],
                    )
                    c += nch

    nc.finalize()
    return nc


def _host_prepare(inputs):
    """Gather embeddings + lay out weights; returns per-core in_maps."""
    bf = ml_dtypes.bfloat16
    wi = np.asarray(inputs["word_idx"]).astype(np.int64)
    pi = np.asarray(inputs["pos_idx"]).astype(np.int64)
    ei = np.asarray(inputs["ext_idx"]).astype(np.int64)
    we = np.asarray(inputs["word_emb"], np.float32)
    pe = np.asarray(inputs["pos_emb"], np.float32)
    xe = np.asarray(inputs["ext_emb"], np.float32)
    vec = np.concatenate([we[wi], pe[pi], xe[ei]], axis=-1)  # [512, 425] f32

    w_ih_f = np.asarray(inputs["w_ih_f"], np.float32)
    w_ih_b = np.asarray(inputs["w_ih_b"], np.float32)
    b_f = np.asarray(inputs["b_f"], np.float32)
    b_b = np.asarray(inputs["b_b"], np.float32)
    w_mlp_in = np.asarray(inputs["w_mlp_in"], np.float32)
    b_mlp_in = np.asarray(inputs["b_mlp_in"], np.float32)
    w_mlp_out = np.asarray(inputs["w_mlp_out"], np.float32)
    b_mlp_out = np.asarray(inputs["b_mlp_out"], np.float32)

    # stacked gate weights [425, 600]: i_f g_f o_f i_b g_b o_b (f unused)
    w6 = np.concatenate(
        [
            w_ih_f[0:100],
            w_ih_f[200:300],
            w_ih_f[300:400],
            w_ih_b[0:100],
            w_ih_b[200:300],
            w_ih_b[300:400],
        ],
        axis=0,
    ).T  # [425, 600]

    bias = np.zeros((100, 11), np.float32)
    for m, sl in enumerate(
        [b_f[0:100], b_f[200:300], b_f[300:400], b_b[0:100], b_b[200:300], b_b[300:400]]
    ):
        bias[:, m] = sl
    bias[:, 6:10] = b_mlp_in.reshape(4, 100).T
    bias[0:NREL, 10] = 0.5 * b_mlp_out

    # row 100: b_out/2 for the natural-layout mlp_out rank-1 bias fold
    # (only the m=3 block's slice is ever read at K=101)
    wo = np.zeros((101, 4 * NREL), np.float32)
    wout_t = w_mlp_out.T  # [400, 42]
    for m in range(4):
        wo[0:100, m * NREL : (m + 1) * NREL] = wout_t[m * 100 : (m + 1) * 100]
        wo[100, m * NREL : (m + 1) * NREL] = 0.5 * b_mlp_out

    # periodic identity block for the pairwise broadcast matmul
    ic = np.zeros((NREL, IC_PER), np.float32)
    cols = np.arange(IC_PER)
    ic[cols % NREL, cols] = 1.0

    def halves(hv):
        if hv:  # cat = [h, vec]
            whx = w_mlp_in[:, 0:200].T  # [200, 400] rows = h features
            wvx = w_mlp_in[:, 200:625].T  # [425, 400] rows = vec features
        else:  # cat = [vec, h]
            whx = w_mlp_in[:, 425:625].T
            wvx = w_mlp_in[:, 0:425].T
        return whx, wvx

    def fill(pk, name, arr):
        _, rows, off, width = SEG[name]
        assert arr.shape == (rows, width), (name, arr.shape, rows, width)
        pk[0:rows, off : off + width] = arr

    in_maps = []
    for core in range(8):
        ib, jh = core // 2, core % 2
        toks = np.concatenate(
            [np.arange(ib * 128, (ib + 1) * 128), np.arange(jh * 256, (jh + 1) * 256)]
        )
        vect = vec[toks].T  # [425, 384]
        g0h, g0v = halves(ib < 2)
        g1h, g1v = halves(jh == 0)

        pk = np.zeros((128, NPK), np.float32)
        for k, (a, b) in enumerate(KS):
            fill(pk, f"vt{k}", vect[a:b])
            fill(pk, f"g6{k}", w6[a:b])
        for g, (gh, gv) in enumerate([(g0h, g0v), (g1h, g1v)]):
            for a in range(2):
                fill(pk, f"wh{g}{a}", gh[a * 100 : (a + 1) * 100])
            for k, (a, b) in enumerate(KS):
                fill(pk, f"wv{g}{k}", gv[a:b])
        fill(pk, "wo", wo)
        fill(pk, "ic", ic)
        in_maps.append(dict(pk=pk.astype(bf), bias=bias))
    return in_maps


_CACHED_NC = None


def kernel(**inputs):
    global _CACHED_NC
    in_maps = _host_prepare(inputs)
    if _CACHED_NC is None:
        _CACHED_NC = _build_program()
    res = run_bass_kernel_spmd(_CACHED_NC, in_maps, list(range(8)))
    full = np.empty((SEQ, SEQ, NREL), np.float32)
    for core in range(8):
        ib, jh = core // 2, core % 2
        blk = res.results[core]["out"].reshape(128, 256, NREL)
        full[ib * 128 : (ib + 1) * 128, jh * 256 : (jh + 1) * 256, :] = blk
    return full


if __name__ == "__main__":
    rng = np.random.default_rng(0)
    demo = dict(
        word_idx=rng.integers(0, 50000, 512),
        pos_idx=rng.integers(0, 48, 512),
        ext_idx=rng.integers(0, 100000, 512),
        word_emb=rng.standard_normal((50000, 100), np.float32) * 0.05,
        pos_emb=rng.standard_normal((48, 25), np.float32) * 0.05,
        ext_emb=rng.standard_normal((100000, 300), np.float32) * 0.05,
        w_ih_f=rng.standard_normal((400, 425), np.float32) * 0.05,
        b_f=rng.standard_normal(400).astype(np.float32) * 0.05,
        w_ih_b=rng.standard_normal((400, 425), np.float32) * 0.05,
        b_b=rng.standard_normal(400).astype(np.float32) * 0.05,
        w_mlp_in=rng.standard_normal((400, 625), np.float32) * 0.05,
        b_mlp_in=rng.standard_normal(400).astype(np.float32) * 0.05,
        w_mlp_out=rng.standard_normal((42, 400), np.float32) * 0.05,
        b_mlp_out=rng.standard_normal(42).astype(np.float32) * 0.05,
    )
    out = kernel(**demo)
    print("out", out.shape, out.dtype, float(np.abs(out).max()))



# revision 5
# speedup vs baseline: 1.1080x; 1.1080x over previous
"""Trainium2 Bass kernel for nn_BiLSTM pairwise-scores problem.

Math (reference):
  vec  = concat(word_emb[wi], pos_emb[pi], ext_emb[ei])          [512, 425]
  h    = concat(lstm_cell_f(vec), lstm_cell_b(vec))              [512, 200]
  cat  = [h, vec] for t <= 255 else [vec, h]                     [512, 625]
  f    = cat @ w_mlp_in.T + b_mlp_in                             [512, 400]
  out  = tanh((f[:,None,:] + f[None,:,:]) @ w_mlp_out.T + b_out) [512, 512, 42]

Two host-side algebraic folds shrink the device program:
  1. mlp_in and mlp_out are both linear, so
       (f_i + f_j) @ Wo.T + b_out = cat_i @ M + cat_j @ M + b''
     with M = W_in.T @ Wo.T  [625, 42].  Each token needs only the tiny
     g' = cat @ M + b'/row projection (b' = b_in @ Wo.T + b_out/2); the
     [625 -> 400] mlp_in stage disappears entirely.
  2. The gate biases ride a ones-row appended to vec (K=426), so the gate
     PSUM already contains w.x+b and the sigmoid/tanh activations batch
     into a few wide ACT instructions with no per-gate bias operands.

Pairwise stage: out[p, j*42+r] = g'_i[p,r] + g'_j[j,r] realized as a
single K=43 matmul per 512-col chunk: lhsT = [g'_iT rows; ones row],
rhs = [periodic identity rows; g'_j flattened row], then one Tanh ACT
per 4-chunk PSUM group, emitted as bf16 (host upcasts to f32).

Sharding: 8 cores = 2 i-halves (256 rows) x 4 j-quarters (128 cols).
Each core runs an identical (SPMD) program on a permuted 384-token slice:
cols 0:256 = its i-half tokens, cols 256:384 = its j-quarter tokens.
Both 128-row i-blocks of a core share one rhs (identity + g'_j flat), so
the identity broadcast is only [42, 5376].
"""

import os
import sys

import numpy as np

for _p in ("/opt/trn_rl_repo", "/root/.axon_site/_ro/trn_rl_repo"):
    if os.path.isdir(_p) and _p not in sys.path:
        sys.path.insert(0, _p)

import ml_dtypes  # noqa: E402

import concourse.bacc as bacc  # noqa: E402
import concourse.bass as bass  # noqa: E402
import concourse.mybir as mybir  # noqa: E402
from concourse.bass_utils import run_bass_kernel_spmd  # noqa: E402
from concourse.tile import TileContext  # noqa: E402

BF16 = mybir.dt.bfloat16
F32 = mybir.dt.float32
AF = mybir.ActivationFunctionType

SEQ = 512
NREL = 42
T = 384          # per-core tokens: 256 (i-half) + 128 (j-quarter)
NI = 256         # i tokens per core
NJ = 128         # j tokens per core
JFLAT = NJ * NREL          # 5376 = per-block output row length
NFLAT = 2 * JFLAT          # 10752 = per-core output row length
IC_PER = 16 * NREL         # 672: replication period for the identity pattern

# K-dim tiling of the 426-dim (vec + ones) feature axis
KS = [(0, 128), (128, 256), (256, 384), (384, 426)]
# gate column order in the stacked [426, 600] gate weight:
# i_f o_f i_b o_b | g_f g_b   (io block first for batched sigmoid ACT)

# ---- packed bf16 constant layout: [128, NPK] ----
_SEGS = []  # name -> (rows, col_off, width)


def _seg(name, rows, width):
    off = _SEGS[-1][2] + _SEGS[-1][3] if _SEGS else 0
    _SEGS.append((name, rows, off, width))


# interleaved (vt_k, g6_k) pairs so the first gate matmuls can start as
# soon as the first small DMA lands
for _k, (_a, _b) in enumerate(KS):
    _seg(f"vt{_k}", _b - _a, T)
    _seg(f"g6{_k}", _b - _a, 600)
# folded mlp weight M [626, 84]: cols 0:42 for the i-half ordering,
# cols 42:84 for the j-quarter ordering (host fills per core).
# rows: vec chunks (426, incl b' row at 425), then h_f (100), h_b (100).
for _k, (_a, _b) in enumerate(KS):
    _seg(f"m{_k}", _b - _a, 2 * NREL)
_seg("mh0", 100, 2 * NREL)
_seg("mh1", 100, 2 * NREL)
_seg("ic", NREL, IC_PER)
SEG = {s[0]: s for s in _SEGS}
NPK = _SEGS[-1][2] + _SEGS[-1][3]
# input DMA split points: one per (vt_k, g6_k) pair, then the remainder
PK_CUTS = [SEG[f"g6{_k}"][2] + SEG[f"g6{_k}"][3] for _k in range(4)] + [NPK]


def _build_program():
    nc = bacc.Bacc()

    pk_d = nc.dram_tensor("pk", [128, NPK], BF16, kind="ExternalInput")
    out_d = nc.dram_tensor("out", [128, NFLAT], BF16, kind="ExternalOutput")

    with TileContext(nc) as tc:
        with (
            tc.tile_pool(name="const", bufs=1) as cp,
            tc.tile_pool(name="work", bufs=1) as wp,
            tc.tile_pool(name="outp", bufs=3) as op_,
        ):
            # -------- early on-chip init (no DMA deps) --------
            wsrc = cp.tile([128, 512], BF16, tag="wsrc")
            nc.gpsimd.memset(wsrc, 0.0)
            # lhsT tiles of the pairwise matmul: rows 0:42 = g'_iT, row 42
            # = 1.0.  DVE partition base must be 32-aligned, so memset
            # 32:43 and let the later g' copy overwrite rows 32:42.
            el = []
            for b in range(2):
                e = cp.tile([NREL + 1, 128], BF16, tag=f"el{b}")
                nc.vector.memset(e[32 : NREL + 1, :], 1.0)
                el.append(e)
            # warmup activations absorb the ACT table-set loads early
            warm2 = cp.tile([1, 8], F32, tag="warm2")
            nc.scalar.activation(out=warm2, in_=wsrc[0:1, 0:8], func=AF.Sigmoid)
            nc.scalar.activation(out=warm2, in_=wsrc[0:1, 0:8], func=AF.Tanh)

            # -------- input DMAs (paired packed chunks) --------
            pk = cp.tile([128, NPK], BF16, tag="pk")
            prev = 0
            for cut in PK_CUTS:
                nc.sync.dma_start(out=pk[:, prev:cut], in_=pk_d[:, prev:cut])
                prev = cut

            def seg(name):
                _, rows, off, width = SEG[name]
                return pk[0:rows, off : off + width]

            vt = [seg(f"vt{k}") for k in range(4)]
            g6 = [seg(f"g6{k}") for k in range(4)]
            mm = [seg(f"m{k}") for k in range(4)] + [seg("mh0"), seg("mh1")]
            ic = seg("ic")

            # pairwise rhs: rows 0:42 = periodic identity, row 42 = g'_j
            # flat.  Both i-blocks share it, so only JFLAT wide.
            rr = cp.tile([NREL + 1, JFLAT], BF16, tag="rr")
            ic_rep = bass.AP(
                tensor=ic.tensor,
                offset=ic.offset,
                ap=[ic.ap[0], [0, JFLAT // IC_PER], ic.ap[1]],
            )
            nc.sync.dma_start(out=rr[0:NREL, :], in_=ic_rep)

            with tc.tile_pool(name="psum_pre", bufs=1, space="PSUM") as pp:
                io_t = pp.tile([128, 2048], F32, tag="io")
                g_t = pp.tile([100, 1024], F32, tag="g")
                gt_t = pp.tile([NREL, NI], F32, tag="gt")
                nat_t = pp.tile([128, NREL], F32, tag="nat")

                # PE warmup: start the HAM busy-window during the DMA wait
                for _ in range(4):
                    nc.tensor.matmul(
                        io_t[:, 0:512],
                        lhsT=wsrc[:, 0:128],
                        rhs=wsrc,
                        start=True,
                        stop=True,
                    )

                # -------- LSTM gates (both dirs, f-gate skipped) --------
                # g gates first: the tanh(g) ACT overlaps the io matmuls.
                def gate(dst, col, ca, cb):
                    for k in range(4):
                        nc.tensor.matmul(
                            dst[0:100, ca:cb],
                            lhsT=g6[k][:, col : col + 100],
                            rhs=vt[k],
                            start=(k == 0),
                            stop=(k == 3),
                        )

                gate(g_t, 400, 0, T)        # g_f
                gate(g_t, 500, 512, 512 + T)  # g_b
                gate(io_t, 0, 0, T)         # i_f
                gate(io_t, 100, 512, 512 + T)  # o_f
                gate(io_t, 200, 1024, 1024 + T)  # i_b
                gate(io_t, 300, 1536, 1536 + T)  # o_b

                def strided_in(tile, base, n):
                    a = tile[0:100, base : base + 512 * n]
                    return bass.AP(
                        tensor=a.tensor,
                        offset=a.offset,
                        ap=[a.ap[0], [512, n], [1, T]],
                    )

                def strided_out(tile, n):
                    a = tile[0:100, 0 : T * n]
                    return bass.AP(
                        tensor=a.tensor,
                        offset=a.offset,
                        ap=[a.ap[0], [T, n], [1, T]],
                    )

                # tanh(g): one ACT over both dirs
                tgs = wp.tile([100, 2 * T], BF16, tag="tgs")
                nc.scalar.activation(
                    out=strided_out(tgs, 2), in_=strided_in(g_t, 0, 2), func=AF.Tanh
                )
                # sigmoid(i), sigmoid(o): one ACT per direction
                sio = []
                for d in range(2):
                    s = wp.tile([100, 2 * T], BF16, tag=f"sio{d}")
                    nc.scalar.activation(
                        out=strided_out(s, 2),
                        in_=strided_in(io_t, 1024 * d, 2),
                        func=AF.Sigmoid,
                    )
                    sio.append(s)
                # c = sig(i) * tanh(g); tanh(c); h = sig(o) * tanh(c)
                cc = wp.tile([100, 2 * T], BF16, tag="cc")
                tcs = wp.tile([100, 2 * T], BF16, tag="tcs")
                hh = []
                for d in range(2):
                    nc.vector.tensor_mul(
                        cc[:, d * T : (d + 1) * T],
                        sio[d][:, 0:T],
                        tgs[:, d * T : (d + 1) * T],
                    )
                    nc.scalar.activation(
                        out=tcs[:, d * T : (d + 1) * T],
                        in_=cc[:, d * T : (d + 1) * T],
                        func=AF.Tanh,
                    )
                    h_ = cp.tile([100, T], BF16, tag=f"h{d}")
                    nc.vector.tensor_mul(
                        h_, sio[d][:, T : 2 * T], tcs[:, d * T : (d + 1) * T]
                    )
                    hh.append(h_)

                # fillers pinned into the ACT/DVE gap: keep the PE activity
                # monitor from re-throttling the clock
                for _ in range(4):
                    nc.tensor.matmul(
                        io_t[:, 0:T],
                        lhsT=sio[0][:, 0:128],
                        rhs=sio[0][:, 0:T],
                        start=True,
                        stop=True,
                    )

                # -------- g' = cat @ M + b': transposed for i, natural
                # for j.  cat chunks: vt0..vt3 (incl ones row), h_f, h_b.
                cat = vt + hh
                for k in range(6):
                    nc.tensor.matmul(
                        gt_t,
                        lhsT=mm[k][:, 0:NREL],
                        rhs=cat[k][:, 0:NI],
                        start=(k == 0),
                        stop=(k == 5),
                    )
                for k in range(6):
                    nc.tensor.matmul(
                        nat_t,
                        lhsT=cat[k][:, NI:T],
                        rhs=mm[k][:, NREL : 2 * NREL],
                        start=(k == 0),
                        stop=(k == 5),
                    )

                # el rows 0:42 <- g'_iT; natural g'_j -> flatten into rr
                for b in range(2):
                    nc.vector.tensor_copy(
                        el[b][0:NREL, :], gt_t[:, b * 128 : (b + 1) * 128]
                    )
                natc = wp.tile([128, NREL], BF16, tag="natc")
                nc.vector.tensor_copy(natc, nat_t)
                nc.sync.dma_start(out=rr[NREL : NREL + 1, :], in_=natc)

            # -------- pairwise: tanh(g'_i + g'_j), bf16 out --------
            # per i-block: 5376 cols = chunks of 512 (+ one 256 tail),
            # grouped (2048, 2048, 1280) per PSUM tile; blocks interleaved.
            groups = []
            for g, (base, cols) in enumerate([(0, 2048), (2048, 2048), (4096, 1280)]):
                for b in range(2):
                    groups.append((b, base, cols))
            with tc.tile_pool(name="psum_pair", bufs=2, space="PSUM") as pq:
                for b, base, cols in groups:
                    ppair = pq.tile([128, 2048], F32, tag="ppair")
                    q = 0
                    while q * 512 < cols:
                        w = min(512, cols - q * 512)
                        nc.tensor.matmul(
                            ppair[:, q * 512 : q * 512 + w],
                            lhsT=el[b],
                            rhs=rr[:, base + q * 512 : base + q * 512 + w],
                            start=True,
                            stop=True,
                        )
                        q += 1
                    ot = op_.tile([128, 2048], BF16, tag="ot")
                    nc.scalar.activation(
                        out=ot[:, 0:cols], in_=ppair[:, 0:cols], func=AF.Tanh
                    )
                    nc.sync.dma_start(
                        out=out_d[:, b * JFLAT + base : b * JFLAT + base + cols],
                        in_=ot[:, 0:cols],
                    )

    nc.finalize()
    return nc


def _host_prepare(inputs):
    """Gather embeddings + fold/lay out weights; returns per-core in_maps."""
    bf = ml_dtypes.bfloat16
    wi = np.asarray(inputs["word_idx"]).astype(np.int64)
    pi = np.asarray(inputs["pos_idx"]).astype(np.int64)
    ei = np.asarray(inputs["ext_idx"]).astype(np.int64)
    we = np.asarray(inputs["word_emb"], np.float32)
    pe = np.asarray(inputs["pos_emb"], np.float32)
    xe = np.asarray(inputs["ext_emb"], np.float32)
    vec = np.concatenate([we[wi], pe[pi], xe[ei]], axis=-1)  # [512, 425] f32

    w_ih_f = np.asarray(inputs["w_ih_f"], np.float32)
    w_ih_b = np.asarray(inputs["w_ih_b"], np.float32)
    b_f = np.asarray(inputs["b_f"], np.float32)
    b_b = np.asarray(inputs["b_b"], np.float32)
    w_mlp_in = np.asarray(inputs["w_mlp_in"], np.float32)
    b_mlp_in = np.asarray(inputs["b_mlp_in"], np.float32)
    w_mlp_out = np.asarray(inputs["w_mlp_out"], np.float32)
    b_mlp_out = np.asarray(inputs["b_mlp_out"], np.float32)

    # stacked gate weights+bias [426, 600]: i_f o_f i_b o_b g_f g_b
    w6 = np.zeros((426, 600), np.float32)
    cols = [
        w_ih_f[0:100], w_ih_f[300:400], w_ih_b[0:100], w_ih_b[300:400],
        w_ih_f[200:300], w_ih_b[200:300],
    ]
    biases = [
        b_f[0:100], b_f[300:400], b_b[0:100], b_b[300:400],
        b_f[200:300], b_b[200:300],
    ]
    for m, (wslab, bslab) in enumerate(zip(cols, biases)):
        w6[0:425, m * 100 : (m + 1) * 100] = wslab.T
        w6[425, m * 100 : (m + 1) * 100] = bslab

    # fold mlp_in into mlp_out: M_raw[k, r] = sum_d Win[d,k] Wout[r,d]
    m_raw = w_mlp_in.T @ w_mlp_out.T          # [625, 42]
    b_half = b_mlp_in @ w_mlp_out.T + 0.5 * b_mlp_out  # [42]

    def m_dev(hv):
        # device row order: vec (425), b' row, h_f (100), h_b (100)
        md = np.zeros((626, NREL), np.float32)
        if hv:   # cat = [h, vec]
            md[0:425] = m_raw[200:625]
            md[426:526] = m_raw[0:100]
            md[526:626] = m_raw[100:200]
        else:    # cat = [vec, h]
            md[0:425] = m_raw[0:425]
            md[426:526] = m_raw[425:525]
            md[526:626] = m_raw[525:625]
        md[425] = b_half
        return md

    m_hv, m_vh = m_dev(True), m_dev(False)

    # periodic identity block for the pairwise broadcast matmul
    ic = np.zeros((NREL, IC_PER), np.float32)
    c = np.arange(IC_PER)
    ic[c % NREL, c] = 1.0

    def fill(pk, name, arr):
        _, rows, off, width = SEG[name]
        assert arr.shape == (rows, width), (name, arr.shape, rows, width)
        pk[0:rows, off : off + width] = arr

    in_maps = []
    for core in range(8):
        ib, jq = core // 4, core % 4
        toks = np.concatenate(
            [np.arange(ib * 256, (ib + 1) * 256), np.arange(jq * 128, (jq + 1) * 128)]
        )
        vect = np.zeros((426, T), np.float32)
        vect[0:425] = vec[toks].T
        vect[425] = 1.0
        m_i = m_hv if ib == 0 else m_vh
        m_j = m_hv if jq < 2 else m_vh
        m2 = np.concatenate([m_i, m_j], axis=1)  # [626, 84]

        pk = np.zeros((128, NPK), np.float32)
        for k, (a, b) in enumerate(KS):
            fill(pk, f"vt{k}", vect[a:b])
            fill(pk, f"g6{k}", w6[a:b])
            fill(pk, f"m{k}", m2[a:b])
        fill(pk, "mh0", m2[426:526])
        fill(pk, "mh1", m2[526:626])
        fill(pk, "ic", ic)
        in_maps.append(dict(pk=pk.astype(bf)))
    return in_maps


_CACHED_NC = None


def kernel(**inputs):
    global _CACHED_NC
    in_maps = _host_prepare(inputs)
    if _CACHED_NC is None:
        _CACHED_NC = _build_program()
    res = run_bass_kernel_spmd(_CACHED_NC, in_maps, list(range(8)))
    full = np.empty((SEQ, SEQ, NREL), np.float32)
    for core in range(8):
        ib, jq = core // 4, core % 4
        blk = np.asarray(res.results[core]["out"], dtype=np.float32)
        for b in range(2):
            full[
                ib * 256 + b * 128 : ib * 256 + (b + 1) * 128,
                jq * 128 : (jq + 1) * 128,
                :,
            ] = blk[:, b * JFLAT : (b + 1) * JFLAT].reshape(128, NJ, NREL)
    return full


if __name__ == "__main__":
    rng = np.random.default_rng(0)
    demo = dict(
        word_idx=rng.integers(0, 50000, 512),
        pos_idx=rng.integers(0, 48, 512),
        ext_idx=rng.integers(0, 100000, 512),
        word_emb=rng.standard_normal((50000, 100), np.float32) * 0.05,
        pos_emb=rng.standard_normal((48, 25), np.float32) * 0.05,
        ext_emb=rng.standard_normal((100000, 300), np.float32) * 0.05,
        w_ih_f=rng.standard_normal((400, 425), np.float32) * 0.05,
        b_f=rng.standard_normal(400).astype(np.float32) * 0.05,
        w_ih_b=rng.standard_normal((400, 425), np.float32) * 0.05,
        b_b=rng.standard_normal(400).astype(np.float32) * 0.05,
        w_mlp_in=rng.standard_normal((400, 625), np.float32) * 0.05,
        b_mlp_in=rng.standard_normal(400).astype(np.float32) * 0.05,
        w_mlp_out=rng.standard_normal((42, 400), np.float32) * 0.05,
        b_mlp_out=rng.standard_normal(42).astype(np.float32) * 0.05,
    )
    out = kernel(**demo)
    print("out", out.shape, out.dtype, float(np.abs(out).max()))


# revision 10
# speedup vs baseline: 1.1840x; 1.0685x over previous
"""Trainium2 Bass kernel for nn_BiLSTM pairwise-scores problem.

Math (reference):
  vec  = concat(word_emb[wi], pos_emb[pi], ext_emb[ei])          [512, 425]
  h    = concat(lstm_cell_f(vec), lstm_cell_b(vec))              [512, 200]
  cat  = [h, vec] for t <= 255 else [vec, h]                     [512, 625]
  f    = cat @ w_mlp_in.T + b_mlp_in                             [512, 400]
  out  = tanh((f[:,None,:] + f[None,:,:]) @ w_mlp_out.T + b_out) [512, 512, 42]

Two host-side algebraic folds shrink the device program:
  1. mlp_in and mlp_out are both linear, so
       (f_i + f_j) @ Wo.T + b_out = cat_i @ M + cat_j @ M + b''
     with M = W_in.T @ Wo.T  [625, 42].  Each token needs only the tiny
     g' = cat @ M + b'/row projection (b' = b_in @ Wo.T + b_out/2); the
     [625 -> 400] mlp_in stage disappears entirely.
  2. The gate biases ride a ones-row appended to vec (K=426), so the gate
     PSUM already contains w.x+b and the sigmoid/tanh activations batch
     into a few wide ACT instructions with no per-gate bias operands.

Pairwise stage: out[p, j*42+r] = g'_i[p,r] + g'_j[j,r] realized as a
single K=43 matmul per 512-col chunk: lhsT = [g'_iT rows; ones row],
rhs = [periodic identity rows; g'_j flattened row], then one Tanh ACT
per 4-chunk PSUM group, emitted as bf16 (host upcasts to f32).

Sharding: 8 cores = 2 i-halves (256 rows) x 4 j-quarters (128 cols).
Each core runs an identical (SPMD) program on a permuted 384-token slice:
cols 0:256 = its i-half tokens, cols 256:384 = its j-quarter tokens.
Both 128-row i-blocks of a core share one rhs (identity + g'_j flat), so
the identity broadcast is only [42, 5376].
"""

import os
import sys

import numpy as np

for _p in ("/opt/trn_rl_repo", "/root/.axon_site/_ro/trn_rl_repo"):
    if os.path.isdir(_p) and _p not in sys.path:
        sys.path.insert(0, _p)

import ml_dtypes  # noqa: E402

import concourse.bacc as bacc  # noqa: E402
import concourse.bass as bass  # noqa: E402
import concourse.mybir as mybir  # noqa: E402
from concourse.bass_utils import run_bass_kernel_spmd  # noqa: E402
from concourse.tile import TileContext  # noqa: E402

BF16 = mybir.dt.bfloat16
F32 = mybir.dt.float32
AF = mybir.ActivationFunctionType

SEQ = 512
NREL = 42
T = 384          # per-core tokens: 256 (i-half) + 128 (j-quarter)
NI = 256         # i tokens per core
NJ = 128         # j tokens per core
JFLAT = NJ * NREL          # 5376 = per-block output row length
NFLAT = 2 * JFLAT          # 10752 = per-core output row length
IC_PER = 8 * NREL          # 336: replication period for the identity pattern

# K-dim tiling of the 426-dim (vec + ones) feature axis
KS = [(0, 128), (128, 256), (256, 384), (384, 426)]
# gate column order in the stacked [426, 600] gate weight:
# i_f o_f i_b o_b | g_f g_b   (io block first for batched sigmoid ACT)

# ---- packed bf16 constant layout: [128, NPK] ----
_SEGS = []  # name -> (rows, col_off, width)


def _seg(name, rows, width):
    off = _SEGS[-1][2] + _SEGS[-1][3] if _SEGS else 0
    _SEGS.append((name, rows, off, width))


# interleaved (vt_k, g6_k) pairs so the first gate matmuls can start as
# soon as the first small DMA lands
for _k, (_a, _b) in enumerate(KS):
    _seg(f"vt{_k}", _b - _a, T)
    _seg(f"g6{_k}", _b - _a, 600)
# folded mlp weight M [626, 84]: cols 0:42 for the i-half ordering,
# cols 42:84 for the j-quarter ordering (host fills per core).
# rows: vec chunks (426, incl b' row at 425), then h_f (100), h_b (100).
for _k, (_a, _b) in enumerate(KS):
    _seg(f"m{_k}", _b - _a, 2 * NREL)
_seg("mh0", 100, 2 * NREL)
_seg("mh1", 100, 2 * NREL)
_seg("ic", NREL, IC_PER)
SEG = {s[0]: s for s in _SEGS}
NPK = _SEGS[-1][2] + _SEGS[-1][3]
# input DMA split points: one per (vt_k, g6_k) pair, then the remainder
PK_CUTS = [SEG[f"g6{_k}"][2] + SEG[f"g6{_k}"][3] for _k in range(4)] + [NPK]


def _build_program():
    nc = bacc.Bacc()

    pk_d = nc.dram_tensor("pk", [128, NPK], BF16, kind="ExternalInput")
    out_d = nc.dram_tensor("out", [128, NFLAT], BF16, kind="ExternalOutput")

    with TileContext(nc) as tc:
        with (
            tc.tile_pool(name="const", bufs=1) as cp,
            tc.tile_pool(name="work", bufs=1) as wp,
            tc.tile_pool(name="outp", bufs=3) as op_,
        ):
            # -------- early on-chip init (no DMA deps) --------
            wsrc = cp.tile([128, 512], BF16, tag="wsrc")
            nc.gpsimd.memset(wsrc, 0.0)
            # lhsT tiles of the pairwise matmul: rows 0:42 = g'_iT, row 42
            # = 1.0.  DVE partition base must be 32-aligned, so memset
            # 32:43 and let the later g' copy overwrite rows 32:42.
            el = []
            for b in range(2):
                e = cp.tile([NREL + 1, 128], BF16, tag=f"el{b}")
                nc.vector.memset(e[32 : NREL + 1, :], 1.0)
                el.append(e)
            # warmup activations absorb the ACT table-set loads early
            warm2 = cp.tile([1, 8], F32, tag="warm2")
            nc.scalar.activation(out=warm2, in_=wsrc[0:1, 0:8], func=AF.Sigmoid)
            nc.scalar.activation(out=warm2, in_=wsrc[0:1, 0:8], func=AF.Tanh)

            # -------- input DMAs: 3 parallel rings --------
            # sync: (vt0,g60)+(vt1,g61); scalar: (vt2,g62)+(vt3,g63);
            # gpsimd (SWDGE): M chunks + ic.  The rings' SDMA queues drain
            # in parallel, so the full input lands in ~1/2 the serial time.
            pk = cp.tile([128, NPK], BF16, tag="pk")
            cuts = [0] + PK_CUTS
            ring = [nc.sync, nc.sync, nc.scalar, nc.scalar, nc.gpsimd]
            for c in range(5):
                ring[c].dma_start(
                    out=pk[:, cuts[c] : cuts[c + 1]], in_=pk_d[:, cuts[c] : cuts[c + 1]]
                )

            def seg(name):
                _, rows, off, width = SEG[name]
                return pk[0:rows, off : off + width]

            vt = [seg(f"vt{k}") for k in range(4)]
            g6 = [seg(f"g6{k}") for k in range(4)]
            mm = [seg(f"m{k}") for k in range(4)] + [seg("mh0"), seg("mh1")]
            ic = seg("ic")

            # pairwise rhs: rows 0:42 = periodic identity, row 42 = g'_j
            # flat.  Both i-blocks share it, so only JFLAT wide.
            rr = cp.tile([NREL + 1, JFLAT], BF16, tag="rr")
            ic_rep = bass.AP(
                tensor=ic.tensor,
                offset=ic.offset,
                ap=[ic.ap[0], [0, JFLAT // IC_PER], ic.ap[1]],
            )
            nc.gpsimd.dma_start(out=rr[0:NREL, :], in_=ic_rep)

            with tc.tile_pool(name="psum_pre", bufs=1, space="PSUM") as pp:
                io_t = pp.tile([128, 2048], F32, tag="io")
                g_t = pp.tile([100, 1024], F32, tag="g")
                gt_t = pp.tile([NREL, NI], F32, tag="gt")
                nat_t = pp.tile([128, NREL], F32, tag="nat")

                # PE warmup: start the HAM busy-window during the DMA wait
                for _ in range(6):
                    nc.tensor.matmul(
                        io_t[:, 0:512],
                        lhsT=wsrc[:, 0:128],
                        rhs=wsrc,
                        start=True,
                        stop=True,
                    )

                # -------- LSTM gates (both dirs, f-gate skipped) --------
                # order g_f g_b i_f i_b o_f o_b: each ACT fires as soon as
                # its pair of gates lands
                def gate(dst, col, ca, cb):
                    for k in range(4):
                        nc.tensor.matmul(
                            dst[0:100, ca:cb],
                            lhsT=g6[k][:, col : col + 100],
                            rhs=vt[k],
                            start=(k == 0),
                            stop=(k == 3),
                        )

                gate(g_t, 400, 0, T)             # g_f
                gate(g_t, 500, 512, 512 + T)     # g_b
                gate(io_t, 0, 0, T)              # i_f
                gate(io_t, 200, 1024, 1024 + T)  # i_b
                gate(io_t, 100, 512, 512 + T)    # o_f
                gate(io_t, 300, 1536, 1536 + T)  # o_b

                def strided_in(tile, base, stride):
                    a = tile[0:100, base : base + stride + T]
                    return bass.AP(
                        tensor=a.tensor,
                        offset=a.offset,
                        ap=[a.ap[0], [stride, 2], [1, T]],
                    )

                def strided_out(tile):
                    a = tile[0:100, 0 : 2 * T]
                    return bass.AP(
                        tensor=a.tensor,
                        offset=a.offset,
                        ap=[a.ap[0], [T, 2], [1, T]],
                    )

                # batched activations: (f,b) pairs in one ACT each
                tgs = wp.tile([100, 2 * T], BF16, tag="tgs")
                nc.scalar.activation(
                    out=strided_out(tgs), in_=strided_in(g_t, 0, 512), func=AF.Tanh
                )
                si = wp.tile([100, 2 * T], BF16, tag="si")
                nc.scalar.activation(
                    out=strided_out(si), in_=strided_in(io_t, 0, 1024), func=AF.Sigmoid
                )
                # c = sig(i) * tanh(g), both dirs in one DVE op
                cc = wp.tile([100, 2 * T], BF16, tag="cc")
                nc.vector.tensor_mul(cc, si, tgs)
                tcs = wp.tile([100, 2 * T], BF16, tag="tcs")
                nc.scalar.activation(out=tcs, in_=cc, func=AF.Tanh)
                so = wp.tile([100, 2 * T], BF16, tag="so")
                nc.scalar.activation(
                    out=strided_out(so), in_=strided_in(io_t, 512, 1024),
                    func=AF.Sigmoid,
                )
                hht = cp.tile([100, 2 * T], BF16, tag="hht")
                nc.vector.tensor_mul(hht, so, tcs)
                hh = [hht[:, 0:T], hht[:, T : 2 * T]]

                # fillers pinned into the ACT/DVE gap: keep the PE activity
                # monitor from re-throttling the clock
                for _ in range(4):
                    nc.tensor.matmul(
                        io_t[:, 0:T],
                        lhsT=si[:, 0:128],
                        rhs=si[:, 0:T],
                        start=True,
                        stop=True,
                    )

                # -------- g' = cat @ M + b': transposed for i, natural
                # for j.  cat chunks: vt0..vt3 (incl ones row), h_f, h_b.
                cat = vt + hh
                for k in range(6):
                    nc.tensor.matmul(
                        gt_t,
                        lhsT=mm[k][:, 0:NREL],
                        rhs=cat[k][:, 0:NI],
                        start=(k == 0),
                        stop=(k == 5),
                    )
                for k in range(6):
                    nc.tensor.matmul(
                        nat_t,
                        lhsT=cat[k][:, NI:T],
                        rhs=mm[k][:, NREL : 2 * NREL],
                        start=(k == 0),
                        stop=(k == 5),
                    )

                # el rows 0:42 <- g'_iT; natural g'_j -> flatten into rr
                for b in range(2):
                    nc.vector.tensor_copy(
                        el[b][0:NREL, :], gt_t[:, b * 128 : (b + 1) * 128]
                    )
                natc = wp.tile([128, NREL], BF16, tag="natc")
                nc.vector.tensor_copy(natc, nat_t)
                nc.scalar.dma_start(out=rr[NREL : NREL + 1, :], in_=natc)

            # -------- pairwise: tanh(g'_i + g'_j), bf16 out --------
            # per i-block: 5376 cols = chunks of 512 (+ one 256 tail),
            # grouped (2048, 2048, 1280) per PSUM tile; blocks interleaved.
            groups = []
            for g, (base, cols) in enumerate([(0, 2048), (2048, 2048), (4096, 1280)]):
                for b in range(2):
                    groups.append((b, base, cols))
            with tc.tile_pool(name="psum_pair", bufs=2, space="PSUM") as pq:
                for b, base, cols in groups:
                    ppair = pq.tile([128, 2048], F32, tag="ppair")
                    q = 0
                    while q * 512 < cols:
                        w = min(512, cols - q * 512)
                        nc.tensor.matmul(
                            ppair[:, q * 512 : q * 512 + w],
                            lhsT=el[b],
                            rhs=rr[:, base + q * 512 : base + q * 512 + w],
                            start=True,
                            stop=True,
                        )
                        q += 1
                    ot = op_.tile([128, 2048], BF16, tag="ot")
                    nc.scalar.activation(
                        out=ot[:, 0:cols], in_=ppair[:, 0:cols], func=AF.Tanh
                    )
                    nc.sync.dma_start(
                        out=out_d[:, b * JFLAT + base : b * JFLAT + base + cols],
                        in_=ot[:, 0:cols],
                    )

    nc.finalize()
    return nc


def _host_prepare(inputs):
    """Gather embeddings + fold/lay out weights; returns per-core in_maps."""
    bf = ml_dtypes.bfloat16
    wi = np.asarray(inputs["word_idx"]).astype(np.int64)
    pi = np.asarray(inputs["pos_idx"]).astype(np.int64)
    ei = np.asarray(inputs["ext_idx"]).astype(np.int64)
    we = np.asarray(inputs["word_emb"], np.float32)
    pe = np.asarray(inputs["pos_emb"], np.float32)
    xe = np.asarray(inputs["ext_emb"], np.float32)
    vec = np.concatenate([we[wi], pe[pi], xe[ei]], axis=-1)  # [512, 425] f32

    w_ih_f = np.asarray(inputs["w_ih_f"], np.float32)
    w_ih_b = np.asarray(inputs["w_ih_b"], np.float32)
    b_f = np.asarray(inputs["b_f"], np.float32)
    b_b = np.asarray(inputs["b_b"], np.float32)
    w_mlp_in = np.asarray(inputs["w_mlp_in"], np.float32)
    b_mlp_in = np.asarray(inputs["b_mlp_in"], np.float32)
    w_mlp_out = np.asarray(inputs["w_mlp_out"], np.float32)
    b_mlp_out = np.asarray(inputs["b_mlp_out"], np.float32)

    # stacked gate weights+bias [426, 600]: i_f o_f i_b o_b g_f g_b
    w6 = np.zeros((426, 600), np.float32)
    cols = [
        w_ih_f[0:100], w_ih_f[300:400], w_ih_b[0:100], w_ih_b[300:400],
        w_ih_f[200:300], w_ih_b[200:300],
    ]
    biases = [
        b_f[0:100], b_f[300:400], b_b[0:100], b_b[300:400],
        b_f[200:300], b_b[200:300],
    ]
    for m, (wslab, bslab) in enumerate(zip(cols, biases)):
        w6[0:425, m * 100 : (m + 1) * 100] = wslab.T
        w6[425, m * 100 : (m + 1) * 100] = bslab

    # fold mlp_in into mlp_out: M_raw[k, r] = sum_d Win[d,k] Wout[r,d]
    m_raw = w_mlp_in.T @ w_mlp_out.T          # [625, 42]
    b_half = b_mlp_in @ w_mlp_out.T + 0.5 * b_mlp_out  # [42]

    def m_dev(hv):
        # device row order: vec (425), b' row, h_f (100), h_b (100)
        md = np.zeros((626, NREL), np.float32)
        if hv:   # cat = [h, vec]
            md[0:425] = m_raw[200:625]
            md[426:526] = m_raw[0:100]
            md[526:626] = m_raw[100:200]
        else:    # cat = [vec, h]
            md[0:425] = m_raw[0:425]
            md[426:526] = m_raw[425:525]
            md[526:626] = m_raw[525:625]
        md[425] = b_half
        return md

    m_hv, m_vh = m_dev(True), m_dev(False)

    # periodic identity block for the pairwise broadcast matmul
    ic = np.zeros((NREL, IC_PER), np.float32)
    c = np.arange(IC_PER)
    ic[c % NREL, c] = 1.0

    def fill(pk, name, arr):
        _, rows, off, width = SEG[name]
        assert arr.shape == (rows, width), (name, arr.shape, rows, width)
        pk[0:rows, off : off + width] = arr

    in_maps = []
    for core in range(8):
        ib, jq = core // 4, core % 4
        toks = np.concatenate(
            [np.arange(ib * 256, (ib + 1) * 256), np.arange(jq * 128, (jq + 1) * 128)]
        )
        vect = np.zeros((426, T), np.float32)
        vect[0:425] = vec[toks].T
        vect[425] = 1.0
        m_i = m_hv if ib == 0 else m_vh
        m_j = m_hv if jq < 2 else m_vh
        m2 = np.concatenate([m_i, m_j], axis=1)  # [626, 84]

        pk = np.zeros((128, NPK), np.float32)
        for k, (a, b) in enumerate(KS):
            fill(pk, f"vt{k}", vect[a:b])
            fill(pk, f"g6{k}", w6[a:b])
            fill(pk, f"m{k}", m2[a:b])
        fill(pk, "mh0", m2[426:526])
        fill(pk, "mh1", m2[526:626])
        fill(pk, "ic", ic)
        in_maps.append(dict(pk=pk.astype(bf)))
    return in_maps


_CACHED_NC = None


def kernel(**inputs):
    global _CACHED_NC
    in_maps = _host_prepare(inputs)
    if _CACHED_NC is None:
        _CACHED_NC = _build_program()
    res = run_bass_kernel_spmd(_CACHED_NC, in_maps, list(range(8)))
    full = np.empty((SEQ, SEQ, NREL), np.float32)
    for core in range(8):
        ib, jq = core // 4, core % 4
        blk = np.asarray(res.results[core]["out"], dtype=np.float32)
        for b in range(2):
            full[
                ib * 256 + b * 128 : ib * 256 + (b + 1) * 128,
                jq * 128 : (jq + 1) * 128,
                :,
            ] = blk[:, b * JFLAT : (b + 1) * JFLAT].reshape(128, NJ, NREL)
    return full


if __name__ == "__main__":
    rng = np.random.default_rng(0)
    demo = dict(
        word_idx=rng.integers(0, 50000, 512),
        pos_idx=rng.integers(0, 48, 512),
        ext_idx=rng.integers(0, 100000, 512),
        word_emb=rng.standard_normal((50000, 100), np.float32) * 0.05,
        pos_emb=rng.standard_normal((48, 25), np.float32) * 0.05,
        ext_emb=rng.standard_normal((100000, 300), np.float32) * 0.05,
        w_ih_f=rng.standard_normal((400, 425), np.float32) * 0.05,
        b_f=rng.standard_normal(400).astype(np.float32) * 0.05,
        w_ih_b=rng.standard_normal((400, 425), np.float32) * 0.05,
        b_b=rng.standard_normal(400).astype(np.float32) * 0.05,
        w_mlp_in=rng.standard_normal((400, 625), np.float32) * 0.05,
        b_mlp_in=rng.standard_normal(400).astype(np.float32) * 0.05,
        w_mlp_out=rng.standard_normal((42, 400), np.float32) * 0.05,
        b_mlp_out=rng.standard_normal(42).astype(np.float32) * 0.05,
    )
    out = kernel(**demo)
    print("out", out.shape, out.dtype, float(np.abs(out).max()))
